# revision 37
# baseline (speedup 1.0000x reference)
"""Multi-head attention (B=2, S=2048, D=1024, H=16, HD=64) on 8 trn2 cores.

Sharding: core c = (b, g) with b = c // 4 (batch), g = c % 4 (group of 4
heads).  Each core computes attention for its 4 heads of its batch and a
partial output projection; the host sums the 4 partials per batch and adds
the bias.

v2 design (vs baseline):
  - all matmul operands bf16 (psum accumulation stays fp32); halves DMA
    and enables FWL weight loads (short diag matmuls become efficient)
  - causal mask added via identity-weight matmul accumulate (PE), not DVE
  - denominator: V tile carries 64 ones-columns so the AV matmul
    replicates the denominator across 64 partitions; normalize =
    reciprocal_approx_fast + one DVE multiply (no gpsimd broadcast)
  - PE warmup matmuls bridge the initial DMA wait (keeps HAM at 2.4GHz)
  - single shared PSUM pair-tile ring for scores/proj/rot/outproj +
    dedicated AV pool; proj/outproj blocks are interleaved into the
    attention stream as PE filler so exp (ACT) lag never idles the PE
"""

import sys

if "/opt/trn_rl_repo" not in sys.path:
    sys.path.insert(0, "/opt/trn_rl_repo")

from collections import deque

import numpy as np
import ml_dtypes

import concourse.bass as bass
import concourse.mybir as mybir
import concourse.tile as tile
from concourse import bacc
from concourse.bass_utils import run_bass_kernel_spmd

FP = mybir.dt.float32
BF = mybir.dt.bfloat16
NPBF = ml_dtypes.bfloat16

B, S, D, H, HD = 2, 2048, 1024, 16, 64
NCORES = 8
GH = 4  # heads per core
GW = GH * HD  # 256 qkv columns / wo rows per core
ST = 512  # s-tile
NST = S // ST
KC = 128  # k-chunk
DCH = D // 128  # 8 contraction chunks
NEG = -1.0e5
NWARM = 12  # PE warmup matmuls during initial DMA


def _emit(nc, tc, xT, wq, wk, wv, wo, cosd, sind, rotm, trim, iden, out, dbg=None):
    Exp = mybir.ActivationFunctionType.Exp
    PS = bass.MemorySpace.PSUM
    with (
        tc.tile_pool(name="const", bufs=1) as cpool,
        tc.tile_pool(name="wts", bufs=1) as wpool,
        tc.tile_pool(name="qkv", bufs=1) as qpool,
        tc.tile_pool(name="xin", bufs=4) as xpool,
        tc.tile_pool(name="wrk", bufs=3) as wrk,
        tc.tile_pool(name="psS", bufs=3, space=PS) as psS,
        tc.tile_pool(name="psA", bufs=2, space=PS) as psA,
    ):
        # ---------------- DMA: weights/x on sync+gpsimd, consts on scalar --
        xTr = xT.ap().rearrange("(c p) s -> p c s", p=128)

        def load_xt(st, q0, q1):
            t = xpool.tile([128, DCH, ST], BF, tag="xt")
            hl = DCH // 2
            ssl = slice(st * ST, (st + 1) * ST)
            q0.dma_start(t[:, 0:hl], xTr[:, 0:hl, ssl])
            q1.dma_start(t[:, hl:DCH], xTr[:, hl:DCH, ssl])
            return t

        # startup-critical tensors split across sync+gpsimd; consts on scalar
        def load_w(dram, nm):
            t = wpool.tile([128, DCH, GW], BF, name=nm, tag=nm)
            nc.sync.dma_start(t[:, 0 : DCH // 2], dram.ap()[:, 0 : DCH // 2])
            nc.gpsimd.dma_start(t[:, DCH // 2 : DCH], dram.ap()[:, DCH // 2 : DCH])
            return t

        wq_sb = load_w(wq, "wq_sb")
        xts = [None] * NST
        xts[0] = load_xt(0, nc.sync, nc.gpsimd)
        wk_sb = load_w(wk, "wk_sb")
        wv_sb = load_w(wv, "wv_sb")
        rot_sb = cpool.tile([128, 128], BF)
        nc.scalar.dma_start(rot_sb[:], rotm.ap())
        cos_sb = cpool.tile([128, S], BF)
        nc.scalar.dma_start(cos_sb[:], cosd.ap())
        sin_sb = cpool.tile([128, S], BF)
        nc.scalar.dma_start(sin_sb[:], sind.ap())
        tri_sb = cpool.tile([128, 128], BF)
        nc.scalar.dma_start(tri_sb[:], trim.ap())
        id_sb = cpool.tile([128, 128], BF)
        nc.scalar.dma_start(id_sb[:], iden.ap())
        xts[1] = load_xt(1, nc.sync, nc.gpsimd)
        wo_sb = wpool.tile([128, 2, D], BF)
        nc.scalar.dma_start(wo_sb[:], wo.ap())
        xts[2] = load_xt(2, nc.sync, nc.gpsimd)
        xts[3] = load_xt(3, nc.sync, nc.gpsimd)

        # ---------------- persistent activations ----------------
        QTt = qpool.tile([128, 2, S], BF)  # roped Q^T (ch = head pair)
        KTt = qpool.tile([128, 2, S], BF)
        Vt = qpool.tile([128, S // KC, GH * KC], BF)  # per head: 64 V | 64 ones
        attT = qpool.tile([128, 2, S], BF)

        warm = cpool.tile([128, ST], BF)
        nc.vector.memset(warm[:], 0.0)
        nc.vector.memset(Vt[:], 1.0)  # ones cols stay 1.0; V halves overwritten

        # ---------------- PE warmup (bridges initial DMA wait) -----------
        def wu(n):
            for _ in range(n):
                wps = psS.tile([128, 2, ST], FP, tag="pair", name="wps")
                nc.tensor.matmul(
                    wps[:, 0, :], warm[:, 0:128], warm[:], start=True, stop=True
                )

        wu(NWARM)

        # ---------------- emitters ----------------
        def proj_qk_ch(wsb, xt, ps, ch):
            for dc in range(DCH):
                nc.tensor.matmul(
                    ps[:, ch, :],
                    wsb[:, dc, ch * 128 : (ch + 1) * 128],
                    xt[:, dc, :],
                    start=(dc == 0),
                    stop=(dc == DCH - 1),
                )

        def rope_raw(ps):
            raw = wrk.tile([128, 2, ST], BF, tag="raw")
            nc.vector.tensor_copy(raw[:], ps[:])
            return raw

        def rope_fin(ps, raw, st, dst):
            ssl = slice(st * ST, (st + 1) * ST)
            t1 = wrk.tile([128, 2, ST], BF, tag="t1")
            for ch in range(2):
                nc.vector.tensor_mul(t1[:, ch], ps[:, ch], cos_sb[:, ssl])
            rps = psS.tile([128, 2, ST], FP, tag="pair")
            for ch in range(2):
                nc.tensor.matmul(rps[:, ch, :], rot_sb[:], raw[:, ch, :], start=True, stop=True)
            t2 = wrk.tile([128, 2, ST], BF, tag="t2")
            for ch in range(2):
                nc.vector.tensor_mul(t2[:, ch], rps[:, ch], sin_sb[:, ssl])
            nc.vector.tensor_add(dst[:, :, ssl], t1[:], t2[:])

        def vproj2(xt, st, tbpair):
            ps = psS.tile([128, 2, ST], FP, tag="pair")
            for i in range(2):
                tb = 2 * tbpair + i
                for dc in range(DCH):
                    nc.tensor.matmul(
                        ps[:, i, 0:GW],
                        xt[:, dc, tb * 128 : (tb + 1) * 128],
                        wv_sb[:, dc, :],
                        start=(dc == 0),
                        stop=(dc == DCH - 1),
                    )
                kc = st * 4 + tb
                src = ps[:, i, 0:GW].rearrange("p (h c) -> p h c", c=HD)
                # ones-first head blocks: cols 0:64 stay 1.0 (denominator rows
                # land on partition base 0 for reciprocal_approx_fast)
                dst = Vt[:, kc].rearrange("p (h c) -> p h c", c=KC)[:, :, HD:KC]
                nc.vector.tensor_copy(dst, src)

        def score_pair(h, qt, kc0):
            ch, r0 = h // 2, (h % 2) * HD
            qs = qt * ST
            sps = psS.tile([128, 2, ST], FP, tag="pair")
            xoffs = []
            for i, kc in enumerate((kc0, kc0 + 1)):
                ks = kc * KC
                off = ks - qs
                xoff = max(0, off)
                xoffs.append(xoff)
                nc.tensor.matmul(
                    sps[:, i, xoff:ST],
                    KTt[r0 : r0 + HD, ch, ks : ks + KC],
                    QTt[r0 : r0 + HD, ch, qs + xoff : qs + ST],
                    start=True,
                    stop=(off < 0),
                )
                if off >= 0:  # diagonal chunk: additive tri mask via PE
                    nc.tensor.matmul(
                        sps[:, i, xoff : xoff + KC],
                        id_sb[:],
                        tri_sb[:],
                        start=False,
                        stop=True,
                    )
            pt = wrk.tile([128, 2, ST], BF, tag="pt")
            s2 = sps[:].rearrange("p a b -> p (a b)")
            p2 = pt[:].rearrange("p a b -> p (a b)")
            nc.scalar.activation(
                p2[:, xoffs[0] : 2 * ST], s2[:, xoffs[0] : 2 * ST], Exp, scale=0.125
            )
            return pt, xoffs

        def av_pair(aps, h, kc0, pt, xoffs, nkc):
            for i, kc in enumerate((kc0, kc0 + 1)):
                xoff = xoffs[i]
                nc.tensor.matmul(
                    aps[:, xoff:ST],
                    Vt[:, kc, h * KC : (h + 1) * KC],
                    pt[:, i, xoff:ST],
                    start=(kc == 0),
                    stop=(kc == nkc - 1),
                )

        def attn_head(st, h, pop):
            qt = st
            qs = qt * ST
            nkc = (qs + ST) // KC
            ch, r0 = h // 2, (h % 2) * HD
            aps = psA.tile([128, ST], FP, tag="av")
            prev = None
            for j, kc0 in enumerate(range(0, nkc, 2)):
                cur = score_pair(h, qt, kc0)
                if prev is not None:
                    av_pair(aps, h, prev[0], prev[1], prev[2], nkc)
                prev = (kc0, cur[0], cur[1])
                if j % 2 == 0:
                    pop()
            av_pair(aps, h, prev[0], prev[1], prev[2], nkc)
            # ones-first Vt blocks: denominator = aps rows 0:63 (base 0),
            # attention values = rows 64:127
            rec = wrk.tile([64, ST], FP, tag="rec")
            nc.vector.reciprocal_approx_fast(rec[:], aps[0:64, :])
            nc.vector.tensor_mul(
                attT[r0 : r0 + HD, ch, qs : qs + ST], aps[64:128, :], rec[:]
            )

        def outproj_qb(qb, tail=False):
            ops = psS.tile([128, 2, ST], FP, tag="pair")
            for half in range(2):
                for ch2 in range(2):
                    nc.tensor.matmul(
                        ops[:, half, :],
                        attT[:, ch2, qb * KC : (qb + 1) * KC],
                        wo_sb[:, ch2, half * ST : (half + 1) * ST],
                        start=(ch2 == 0),
                        stop=(ch2 == 1),
                    )
            ob = wrk.tile([128, 2, ST], BF, tag="ob")
            if tail and qb % 2:  # tail: split evac across ACT+DVE
                nc.scalar.copy(ob[:], ops[:])
            else:
                nc.vector.tensor_copy(ob[:], ops[:])
            q = nc.gpsimd if qb % 2 else nc.sync
            q.dma_start(
                out[qb * KC : (qb + 1) * KC, :], ob[:].rearrange("p a b -> p (a b)")
            )

        def proj_blocks(st):
            xt = xts[st]
            ps_q, ps_k = [None], [None]
            raw_q, raw_k = [None], [None]

            def qk_first(ps_box, wsb):
                def f():
                    ps_box[0] = psS.tile([128, 2, ST], FP, tag="pair", name="ps_qk")
                    proj_qk_ch(wsb, xt, ps_box[0], 0)

                return f

            def qk_second(ps_box, wsb, raw_box):
                def f():
                    proj_qk_ch(wsb, xt, ps_box[0], 1)
                    raw_box[0] = rope_raw(ps_box[0])  # ACT copy issues early

                return f

            # raw copy (ACT) is emitted with the ch1 matmuls; the rot matmul
            # block comes two blocks later so the PE never waits on ACT.
            return [
                qk_first(ps_q, wq_sb),
                qk_second(ps_q, wq_sb, raw_q),
                qk_first(ps_k, wk_sb),
                lambda: rope_fin(ps_q[0], raw_q[0], st, QTt),
                qk_second(ps_k, wk_sb, raw_k),
                lambda: vproj2(xt, st, 0),
                lambda: rope_fin(ps_k[0], raw_k[0], st, KTt),
                lambda: vproj2(xt, st, 1),
            ]

        # ---------------- schedule ----------------
        # prologue: P(0) straight, then A(st) with interleaved fillers:
        #   A(0): P(1);  A(1): P(2)+O(0);  A(2): P(3)+O(1);  A(3): O(2)
        # epilogue: O(3)
        # P(0) with warmup matmuls masking the wk/wv DMA arrival
        pb0 = proj_blocks(0)
        pb0[0]()  # Q ch0
        pb0[1]()  # Q ch1 + raw_q
        wu(4)
        pb0[2]()  # K ch0
        pb0[3]()  # rope_fin q
        wu(4)
        for blk in pb0[4:]:
            blk()

        for st in range(NST):
            fillers = deque()
            pblk = proj_blocks(st + 1) if st + 1 < NST else []
            # outproj placement: A(1) gets O(0); A(3) gets O(1)+O(2) — st3 has
            # no projection filler left and is the most exp-paced, so it needs
            # the extra PE work to ride out the ACTIVATE overhead.
            if st == 1:
                oqbs = range(0, 4)
            elif st == 3:
                oqbs = range(4, 12)
            else:
                oqbs = range(0)
            oblk = [lambda qb=qb: outproj_qb(qb) for qb in oqbs]
            # interleave: two proj blocks, then one outproj block
            pi = oi = 0
            while pi < len(pblk) or oi < len(oblk):
                for _ in range(2):
                    if pi < len(pblk):
                        fillers.append(pblk[pi])
                        pi += 1
                if oi < len(oblk):
                    fillers.append(oblk[oi])
                    oi += 1

            def pop():
                if fillers:
                    fillers.popleft()()

            for h in range(GH):
                attn_head(st, h, pop)
                pop()
            while fillers:
                fillers.popleft()()

        for qb in range((NST - 1) * 4, NST * 4):
            outproj_qb(qb, tail=True)

        if dbg is not None:
            nc.sync.dma_start(dbg["QTt"].ap(), QTt[:].rearrange("p a b -> p (a b)"))
            nc.sync.dma_start(dbg["KTt"].ap(), KTt[:].rearrange("p a b -> p (a b)"))
            nc.sync.dma_start(dbg["Vt"].ap(), Vt[:].rearrange("p a b -> p (a b)"))
            nc.sync.dma_start(dbg["attT"].ap(), attT[:].rearrange("p a b -> p (a b)"))


_prog = None


def _build(with_dbg=False):
    global _prog
    if _prog is not None and not with_dbg:
        return _prog
    nc = bacc.Bacc("TRN2", target_bir_lowering=False, debug=False)
    xT = nc.declare_dram_parameter("xT", [D, S], BF, isOutput=False)
    wq = nc.declare_dram_parameter("wq", [128, DCH, GW], BF, isOutput=False)
    wk = nc.declare_dram_parameter("wk", [128, DCH, GW], BF, isOutput=False)
    wv = nc.declare_dram_parameter("wv", [128, DCH, GW], BF, isOutput=False)
    wo = nc.declare_dram_parameter("wo", [128, 2, D], BF, isOutput=False)
    cosd = nc.declare_dram_parameter("cosd", [128, S], BF, isOutput=False)
    sind = nc.declare_dram_parameter("sind", [128, S], BF, isOutput=False)
    rotm = nc.declare_dram_parameter("rotm", [128, 128], BF, isOutput=False)
    trim = nc.declare_dram_parameter("trim", [128, 128], BF, isOutput=False)
    iden = nc.declare_dram_parameter("iden", [128, 128], BF, isOutput=False)
    out = nc.declare_dram_parameter("out", [S, D], BF, isOutput=True)
    dbg = None
    if with_dbg:
        dbg = {
            "QTt": nc.declare_dram_parameter("dbg_QTt", [128, 2 * S], BF, isOutput=True),
            "KTt": nc.declare_dram_parameter("dbg_KTt", [128, 2 * S], BF, isOutput=True),
            "Vt": nc.declare_dram_parameter("dbg_Vt", [128, (S // KC) * GH * KC], BF, isOutput=True),
            "attT": nc.declare_dram_parameter("dbg_attT", [128, 2 * S], BF, isOutput=True),
        }
    with tile.TileContext(nc) as tc:
        _emit(nc, tc, xT, wq, wk, wv, wo, cosd, sind, rotm, trim, iden, out, dbg)
    nc.compile()
    if not with_dbg:
        _prog = nc
    return nc


def _tables():
    inv = 1.0 / (10000.0 ** (np.arange(0, HD, 2)[: HD // 2].astype(np.float32) / HD))
    ang = np.outer(np.arange(S, dtype=np.float32), inv).astype(np.float32)  # [S, 32]
    cos64 = np.repeat(np.cos(ang).T, 2, axis=0).astype(np.float32)  # [64, S]
    sin64 = np.repeat(np.sin(ang).T, 2, axis=0).astype(np.float32)
    cos128 = np.tile(cos64, (2, 1))
    sin128 = np.tile(sin64, (2, 1))
    rotm = np.zeros((128, 128), np.float32)
    for f in range(64):
        rotm[2 * f + 1, 2 * f] = -1.0  # out[2f]   = -x[2f+1]
        rotm[2 * f, 2 * f + 1] = 1.0  # out[2f+1] = +x[2f]
    kk, qq = np.meshgrid(np.arange(128), np.arange(128), indexing="ij")
    tri = np.where(kk <= qq, 0.0, NEG).astype(np.float32)
    iden = np.eye(128, dtype=np.float32)
    return (
        cos128.astype(NPBF),
        sin128.astype(NPBF),
        rotm.astype(NPBF),
        tri.astype(NPBF),
        iden.astype(NPBF),
    )


def _pack_w(w):  # [D, GW] -> [128, DCH, GW] bf16
    return np.ascontiguousarray(
        np.asarray(w, np.float32).reshape(DCH, 128, GW).transpose(1, 0, 2)
    ).astype(NPBF)


def _pack_wo(w):  # [GW, D] -> [128, 2, D] bf16
    return np.ascontiguousarray(
        np.asarray(w, np.float32).reshape(2, 128, D).transpose(1, 0, 2)
    ).astype(NPBF)


def make_in_maps(x, wq, wk, wv, wo_w):
    cos128, sin128, rotm, tri, iden = _tables()
    in_maps = []
    for c in range(NCORES):
        b, g = divmod(c, GH)
        cs = slice(g * GW, (g + 1) * GW)
        in_maps.append(
            {
                "xT": np.ascontiguousarray(np.asarray(x[b], np.float32).T).astype(NPBF),
                "wq": _pack_w(wq[:, cs]),
                "wk": _pack_w(wk[:, cs]),
                "wv": _pack_w(wv[:, cs]),
                "wo": _pack_wo(wo_w[cs, :]),
                "cosd": cos128,
                "sind": sin128,
                "rotm": rotm,
                "trim": tri,
                "iden": iden,
            }
        )
    return in_maps


def kernel(x, wq, wk, wv, wo_w, wo_b):
    nc = _build()
    in_maps = make_in_maps(x, wq, wk, wv, wo_w)
    res = run_bass_kernel_spmd(nc, in_maps, list(range(NCORES))).results
    out = np.zeros((B, S, D), np.float32)
    for c in range(NCORES):
        out[c // GH] += np.asarray(res[c]["out"]).astype(np.float32)
    out += np.asarray(wo_b, np.float32)[None, None, :]
    return out


# revision 41
# speedup vs baseline: 1.0194x; 1.0194x over previous
"""Multi-head attention (B=2, S=2048, D=1024, H=16, HD=64) on 8 trn2 cores.

Sharding: core c = (b, g) with b = c // 4 (batch), g = c % 4 (group of 4
heads).  Each core computes attention for its 4 heads of its batch and a
partial output projection; the host sums the 4 partials per batch and adds
the bias.

v2 design (vs baseline):
  - all matmul operands bf16 (psum accumulation stays fp32); halves DMA
    and enables FWL weight loads (short diag matmuls become efficient)
  - causal mask added via identity-weight matmul accumulate (PE), not DVE
  - denominator: V tile carries 64 ones-columns so the AV matmul
    replicates the denominator across 64 partitions; normalize =
    reciprocal_approx_fast + one DVE multiply (no gpsimd broadcast)
  - PE warmup matmuls bridge the initial DMA wait (keeps HAM at 2.4GHz)
  - single shared PSUM pair-tile ring for scores/proj/rot/outproj +
    dedicated AV pool; proj/outproj blocks are interleaved into the
    attention stream as PE filler so exp (ACT) lag never idles the PE
"""

import sys

if "/opt/trn_rl_repo" not in sys.path:
    sys.path.insert(0, "/opt/trn_rl_repo")

from collections import deque

import numpy as np
import ml_dtypes

import concourse.bass as bass
import concourse.mybir as mybir
import concourse.tile as tile
from concourse import bacc
from concourse.bass_utils import run_bass_kernel_spmd

FP = mybir.dt.float32
BF = mybir.dt.bfloat16
NPBF = ml_dtypes.bfloat16

B, S, D, H, HD = 2, 2048, 1024, 16, 64
NCORES = 8
GH = 4  # heads per core
GW = GH * HD  # 256 qkv columns / wo rows per core
ST = 512  # s-tile
NST = S // ST
KC = 128  # k-chunk
DCH = D // 128  # 8 contraction chunks
NEG = -1.0e5
NWARM = 12  # PE warmup matmuls during initial DMA


def _emit(nc, tc, xT, wq, wk, wv, wo, cosd, sind, rotm, trim, iden, out, dbg=None):
    Exp = mybir.ActivationFunctionType.Exp
    PS = bass.MemorySpace.PSUM
    with (
        tc.tile_pool(name="const", bufs=1) as cpool,
        tc.tile_pool(name="wts", bufs=1) as wpool,
        tc.tile_pool(name="qkv", bufs=1) as qpool,
        tc.tile_pool(name="xin", bufs=4) as xpool,
        tc.tile_pool(name="wrk", bufs=3) as wrk,
        tc.tile_pool(name="psS", bufs=3, space=PS) as psS,
        tc.tile_pool(name="psA", bufs=2, space=PS) as psA,
    ):
        # ---------------- DMA: weights/x on sync+gpsimd, consts on scalar --
        xTr = xT.ap().rearrange("(c p) s -> p c s", p=128)

        def load_xt(st, q0, q1):
            t = xpool.tile([128, DCH, ST], BF, tag="xt")
            hl = DCH // 2
            ssl = slice(st * ST, (st + 1) * ST)
            q0.dma_start(t[:, 0:hl], xTr[:, 0:hl, ssl])
            q1.dma_start(t[:, hl:DCH], xTr[:, hl:DCH, ssl])
            return t

        # startup-critical tensors split across sync+gpsimd; consts on scalar
        def load_w(dram, nm):
            t = wpool.tile([128, DCH, GW], BF, name=nm, tag=nm)
            nc.sync.dma_start(t[:, 0 : DCH // 2], dram.ap()[:, 0 : DCH // 2])
            nc.gpsimd.dma_start(t[:, DCH // 2 : DCH], dram.ap()[:, DCH // 2 : DCH])
            return t

        wq_sb = load_w(wq, "wq_sb")
        xts = [None] * NST
        xts[0] = load_xt(0, nc.sync, nc.gpsimd)
        wk_sb = load_w(wk, "wk_sb")
        wv_sb = load_w(wv, "wv_sb")
        rot_sb = cpool.tile([128, 128], BF)
        nc.scalar.dma_start(rot_sb[:], rotm.ap())
        cos_sb = cpool.tile([128, S], BF)
        nc.scalar.dma_start(cos_sb[:], cosd.ap())
        sin_sb = cpool.tile([128, S], BF)
        nc.scalar.dma_start(sin_sb[:], sind.ap())
        tri_sb = cpool.tile([128, 128], BF)
        nc.scalar.dma_start(tri_sb[:], trim.ap())
        id_sb = cpool.tile([128, 128], BF)
        nc.scalar.dma_start(id_sb[:], iden.ap())
        xts[1] = load_xt(1, nc.sync, nc.gpsimd)
        wo_sb = wpool.tile([128, 2, D], BF)
        nc.scalar.dma_start(wo_sb[:], wo.ap())
        xts[2] = load_xt(2, nc.sync, nc.gpsimd)
        xts[3] = load_xt(3, nc.sync, nc.gpsimd)

        # ---------------- persistent activations ----------------
        QTt = qpool.tile([128, 2, S], BF)  # roped Q^T (ch = head pair)
        KTt = qpool.tile([128, 2, S], BF)
        Vt = qpool.tile([128, S // KC, GH * KC], BF)  # per head: 64 V | 64 ones
        attT = qpool.tile([128, 2, S], BF)

        warm = cpool.tile([128, ST], BF)
        nc.vector.memset(warm[:], 0.0)
        nc.vector.memset(Vt[:], 1.0)  # ones cols stay 1.0; V halves overwritten

        # ---------------- PE warmup (bridges initial DMA wait) -----------
        def wu(n):
            for _ in range(n):
                wps = psS.tile([128, 2, ST], FP, tag="pair", name="wps")
                nc.tensor.matmul(
                    wps[:, 0, :], warm[:, 0:128], warm[:], start=True, stop=True
                )

        wu(NWARM)

        # ---------------- emitters ----------------
        def proj_qk_ch(wsb, xt, ps, ch):
            for dc in range(DCH):
                nc.tensor.matmul(
                    ps[:, ch, :],
                    wsb[:, dc, ch * 128 : (ch + 1) * 128],
                    xt[:, dc, :],
                    start=(dc == 0),
                    stop=(dc == DCH - 1),
                )

        def rope_raw(ps):
            raw = wrk.tile([128, 2, ST], BF, tag="raw")
            nc.vector.tensor_copy(raw[:], ps[:])
            return raw

        def rope_fin(ps, raw, st, dst):
            ssl = slice(st * ST, (st + 1) * ST)
            t1 = wrk.tile([128, 2, ST], BF, tag="t1")
            for ch in range(2):
                nc.vector.tensor_mul(t1[:, ch], ps[:, ch], cos_sb[:, ssl])
            rps = psS.tile([128, 2, ST], FP, tag="pair")
            for ch in range(2):
                nc.tensor.matmul(rps[:, ch, :], rot_sb[:], raw[:, ch, :], start=True, stop=True)
            t2 = wrk.tile([128, 2, ST], BF, tag="t2")
            for ch in range(2):
                nc.vector.tensor_mul(t2[:, ch], rps[:, ch], sin_sb[:, ssl])
            nc.vector.tensor_add(dst[:, :, ssl], t1[:], t2[:])

        def vproj2(xt, st, tbpair):
            ps = psS.tile([128, 2, ST], FP, tag="pair")
            for i in range(2):
                tb = 2 * tbpair + i
                for dc in range(DCH):
                    nc.tensor.matmul(
                        ps[:, i, 0:GW],
                        xt[:, dc, tb * 128 : (tb + 1) * 128],
                        wv_sb[:, dc, :],
                        start=(dc == 0),
                        stop=(dc == DCH - 1),
                    )
                kc = st * 4 + tb
                src = ps[:, i, 0:GW].rearrange("p (h c) -> p h c", c=HD)
                # ones-first head blocks: cols 0:64 stay 1.0 (denominator rows
                # land on partition base 0 for reciprocal_approx_fast)
                dst = Vt[:, kc].rearrange("p (h c) -> p h c", c=KC)[:, :, HD:KC]
                nc.vector.tensor_copy(dst, src)

        def score_pair(h, qt, kc0):
            ch, r0 = h // 2, (h % 2) * HD
            qs = qt * ST
            sps = psS.tile([128, 2, ST], FP, tag="pair")
            xoffs = []
            for i, kc in enumerate((kc0, kc0 + 1)):
                ks = kc * KC
                off = ks - qs
                xoff = max(0, off)
                xoffs.append(xoff)
                nc.tensor.matmul(
                    sps[:, i, xoff:ST],
                    KTt[r0 : r0 + HD, ch, ks : ks + KC],
                    QTt[r0 : r0 + HD, ch, qs + xoff : qs + ST],
                    start=True,
                    stop=(off < 0),
                )
                if off >= 0:  # diagonal chunk: additive tri mask via PE
                    nc.tensor.matmul(
                        sps[:, i, xoff : xoff + KC],
                        id_sb[:],
                        tri_sb[:],
                        start=False,
                        stop=True,
                    )
            pt = wrk.tile([128, 2, ST], BF, tag="pt")
            s2 = sps[:].rearrange("p a b -> p (a b)")
            p2 = pt[:].rearrange("p a b -> p (a b)")
            nc.scalar.activation(
                p2[:, xoffs[0] : 2 * ST], s2[:, xoffs[0] : 2 * ST], Exp, scale=0.125
            )
            return pt, xoffs

        def av_pair(aps, h, kc0, pt, xoffs, nkc):
            for i, kc in enumerate((kc0, kc0 + 1)):
                xoff = xoffs[i]
                nc.tensor.matmul(
                    aps[:, xoff:ST],
                    Vt[:, kc, h * KC : (h + 1) * KC],
                    pt[:, i, xoff:ST],
                    start=(kc == 0),
                    stop=(kc == nkc - 1),
                )

        def attn_head(st, h, pop, quota):
            qt = st
            qs = qt * ST
            nkc = (qs + ST) // KC
            ch, r0 = h // 2, (h % 2) * HD
            aps = psA.tile([128, ST], FP, tag="av")
            prev = None
            cnt = 0
            for j, kc0 in enumerate(range(0, nkc, 2)):
                cur = score_pair(h, qt, kc0)
                if prev is not None:
                    av_pair(aps, h, prev[0], prev[1], prev[2], nkc)
                prev = (kc0, cur[0], cur[1])
                if j % 2 == 1 and cnt < quota:
                    pop()
                    cnt += 1
            av_pair(aps, h, prev[0], prev[1], prev[2], nkc)
            # ones-first Vt blocks: denominator = aps rows 0:63 (base 0),
            # attention values = rows 64:127
            rec = wrk.tile([64, ST], FP, tag="rec")
            nc.vector.reciprocal_approx_fast(rec[:], aps[0:64, :])
            nc.vector.tensor_mul(
                attT[r0 : r0 + HD, ch, qs : qs + ST], aps[64:128, :], rec[:]
            )
            while cnt < quota:  # leftover filler lands after the finish chain
                pop()
                cnt += 1

        def outproj_qb(qb, tail=False):
            ops = psS.tile([128, 2, ST], FP, tag="pair")
            for half in range(2):
                for ch2 in range(2):
                    nc.tensor.matmul(
                        ops[:, half, :],
                        attT[:, ch2, qb * KC : (qb + 1) * KC],
                        wo_sb[:, ch2, half * ST : (half + 1) * ST],
                        start=(ch2 == 0),
                        stop=(ch2 == 1),
                    )
            ob = wrk.tile([128, 2, ST], BF, tag="ob")
            if tail and qb % 2:  # tail: split evac across ACT+DVE
                nc.scalar.copy(ob[:], ops[:])
            else:
                nc.vector.tensor_copy(ob[:], ops[:])
            q = nc.gpsimd if qb % 2 else nc.sync
            q.dma_start(
                out[qb * KC : (qb + 1) * KC, :], ob[:].rearrange("p a b -> p (a b)")
            )

        def proj_blocks(st):
            xt = xts[st]
            ps_q, ps_k = [None], [None]
            raw_q, raw_k = [None], [None]

            def qk_first(ps_box, wsb):
                def f():
                    ps_box[0] = psS.tile([128, 2, ST], FP, tag="pair", name="ps_qk")
                    proj_qk_ch(wsb, xt, ps_box[0], 0)

                return f

            def qk_second(ps_box, wsb, raw_box):
                def f():
                    proj_qk_ch(wsb, xt, ps_box[0], 1)
                    raw_box[0] = rope_raw(ps_box[0])  # ACT copy issues early

                return f

            # raw copy (ACT) is emitted with the ch1 matmuls; the rot matmul
            # block comes two blocks later so the PE never waits on ACT.
            return [
                qk_first(ps_q, wq_sb),
                qk_second(ps_q, wq_sb, raw_q),
                qk_first(ps_k, wk_sb),
                lambda: rope_fin(ps_q[0], raw_q[0], st, QTt),
                qk_second(ps_k, wk_sb, raw_k),
                lambda: vproj2(xt, st, 0),
                lambda: rope_fin(ps_k[0], raw_k[0], st, KTt),
                lambda: vproj2(xt, st, 1),
            ]

        # ---------------- schedule ----------------
        # prologue: P(0) straight, then A(st) with interleaved fillers:
        #   A(0): P(1);  A(1): P(2)+O(0);  A(2): P(3)+O(1);  A(3): O(2)
        # epilogue: O(3)
        # P(0) with warmup matmuls masking the wk/wv DMA arrival
        pb0 = proj_blocks(0)
        pb0[0]()  # Q ch0
        pb0[1]()  # Q ch1 + raw_q
        wu(4)
        pb0[2]()  # K ch0
        pb0[3]()  # rope_fin q
        wu(4)
        for blk in pb0[4:]:
            blk()

        for st in range(NST):
            fillers = deque()
            pblk = proj_blocks(st + 1) if st + 1 < NST else []
            oblk = (
                [lambda qb=qb: outproj_qb(qb) for qb in range((st - 1) * 4, st * 4)]
                if st >= 1
                else []
            )
            # interleave: two proj blocks, then one outproj block
            pi = oi = 0
            while pi < len(pblk) or oi < len(oblk):
                for _ in range(2):
                    if pi < len(pblk):
                        fillers.append(pblk[pi])
                        pi += 1
                if oi < len(oblk):
                    fillers.append(oblk[oi])
                    oi += 1

            def pop():
                if fillers:
                    fillers.popleft()()

            # even per-head filler quotas: FIFO-on-demand starves the last
            # heads of each s-tile (h3 got zero filler and ran exp-paced)
            nf = len(fillers)
            for h in range(GH):
                q = (nf * (h + 1)) // GH - (nf * h) // GH
                attn_head(st, h, pop, q)
            while fillers:
                fillers.popleft()()

        for qb in range((NST - 1) * 4, NST * 4):
            outproj_qb(qb, tail=True)

        if dbg is not None:
            nc.sync.dma_start(dbg["QTt"].ap(), QTt[:].rearrange("p a b -> p (a b)"))
            nc.sync.dma_start(dbg["KTt"].ap(), KTt[:].rearrange("p a b -> p (a b)"))
            nc.sync.dma_start(dbg["Vt"].ap(), Vt[:].rearrange("p a b -> p (a b)"))
            nc.sync.dma_start(dbg["attT"].ap(), attT[:].rearrange("p a b -> p (a b)"))


_prog = None


def _build(with_dbg=False):
    global _prog
    if _prog is not None and not with_dbg:
        return _prog
    nc = bacc.Bacc("TRN2", target_bir_lowering=False, debug=False)
    xT = nc.declare_dram_parameter("xT", [D, S], BF, isOutput=False)
    wq = nc.declare_dram_parameter("wq", [128, DCH, GW], BF, isOutput=False)
    wk = nc.declare_dram_parameter("wk", [128, DCH, GW], BF, isOutput=False)
    wv = nc.declare_dram_parameter("wv", [128, DCH, GW], BF, isOutput=False)
    wo = nc.declare_dram_parameter("wo", [128, 2, D], BF, isOutput=False)
    cosd = nc.declare_dram_parameter("cosd", [128, S], BF, isOutput=False)
    sind = nc.declare_dram_parameter("sind", [128, S], BF, isOutput=False)
    rotm = nc.declare_dram_parameter("rotm", [128, 128], BF, isOutput=False)
    trim = nc.declare_dram_parameter("trim", [128, 128], BF, isOutput=False)
    iden = nc.declare_dram_parameter("iden", [128, 128], BF, isOutput=False)
    out = nc.declare_dram_parameter("out", [S, D], BF, isOutput=True)
    dbg = None
    if with_dbg:
        dbg = {
            "QTt": nc.declare_dram_parameter("dbg_QTt", [128, 2 * S], BF, isOutput=True),
            "KTt": nc.declare_dram_parameter("dbg_KTt", [128, 2 * S], BF, isOutput=True),
            "Vt": nc.declare_dram_parameter("dbg_Vt", [128, (S // KC) * GH * KC], BF, isOutput=True),
            "attT": nc.declare_dram_parameter("dbg_attT", [128, 2 * S], BF, isOutput=True),
        }
    with tile.TileContext(nc) as tc:
        _emit(nc, tc, xT, wq, wk, wv, wo, cosd, sind, rotm, trim, iden, out, dbg)
    nc.compile()
    if not with_dbg:
        _prog = nc
    return nc


def _tables():
    inv = 1.0 / (10000.0 ** (np.arange(0, HD, 2)[: HD // 2].astype(np.float32) / HD))
    ang = np.outer(np.arange(S, dtype=np.float32), inv).astype(np.float32)  # [S, 32]
    cos64 = np.repeat(np.cos(ang).T, 2, axis=0).astype(np.float32)  # [64, S]
    sin64 = np.repeat(np.sin(ang).T, 2, axis=0).astype(np.float32)
    cos128 = np.tile(cos64, (2, 1))
    sin128 = np.tile(sin64, (2, 1))
    rotm = np.zeros((128, 128), np.float32)
    for f in range(64):
        rotm[2 * f + 1, 2 * f] = -1.0  # out[2f]   = -x[2f+1]
        rotm[2 * f, 2 * f + 1] = 1.0  # out[2f+1] = +x[2f]
    kk, qq = np.meshgrid(np.arange(128), np.arange(128), indexing="ij")
    tri = np.where(kk <= qq, 0.0, NEG).astype(np.float32)
    iden = np.eye(128, dtype=np.float32)
    return (
        cos128.astype(NPBF),
        sin128.astype(NPBF),
        rotm.astype(NPBF),
        tri.astype(NPBF),
        iden.astype(NPBF),
    )


def _pack_w(w):  # [D, GW] -> [128, DCH, GW] bf16
    return np.ascontiguousarray(
        np.asarray(w, np.float32).reshape(DCH, 128, GW).transpose(1, 0, 2)
    ).astype(NPBF)


def _pack_wo(w):  # [GW, D] -> [128, 2, D] bf16
    return np.ascontiguousarray(
        np.asarray(w, np.float32).reshape(2, 128, D).transpose(1, 0, 2)
    ).astype(NPBF)


def make_in_maps(x, wq, wk, wv, wo_w):
    cos128, sin128, rotm, tri, iden = _tables()
    in_maps = []
    for c in range(NCORES):
        b, g = divmod(c, GH)
        cs = slice(g * GW, (g + 1) * GW)
        in_maps.append(
            {
                "xT": np.ascontiguousarray(np.asarray(x[b], np.float32).T).astype(NPBF),
                "wq": _pack_w(wq[:, cs]),
                "wk": _pack_w(wk[:, cs]),
                "wv": _pack_w(wv[:, cs]),
                "wo": _pack_wo(wo_w[cs, :]),
                "cosd": cos128,
                "sind": sin128,
                "rotm": rotm,
                "trim": tri,
                "iden": iden,
            }
        )
    return in_maps


def kernel(x, wq, wk, wv, wo_w, wo_b):
    nc = _build()
    in_maps = make_in_maps(x, wq, wk, wv, wo_w)
    res = run_bass_kernel_spmd(nc, in_maps, list(range(NCORES))).results
    out = np.zeros((B, S, D), np.float32)
    for c in range(NCORES):
        out[c // GH] += np.asarray(res[c]["out"]).astype(np.float32)
    out += np.asarray(wo_b, np.float32)[None, None, :]
    return out


# revision 42
# speedup vs baseline: 1.0212x; 1.0018x over previous
"""Multi-head attention (B=2, S=2048, D=1024, H=16, HD=64) on 8 trn2 cores.

Sharding: core c = (b, g) with b = c // 4 (batch), g = c % 4 (group of 4
heads).  Each core computes attention for its 4 heads of its batch and a
partial output projection; the host sums the 4 partials per batch and adds
the bias.

v2 design (vs baseline):
  - all matmul operands bf16 (psum accumulation stays fp32); halves DMA
    and enables FWL weight loads (short diag matmuls become efficient)
  - causal mask added via identity-weight matmul accumulate (PE), not DVE
  - denominator: V tile carries 64 ones-columns so the AV matmul
    replicates the denominator across 64 partitions; normalize =
    reciprocal_approx_fast + one DVE multiply (no gpsimd broadcast)
  - PE warmup matmuls bridge the initial DMA wait (keeps HAM at 2.4GHz)
  - single shared PSUM pair-tile ring for scores/proj/rot/outproj +
    dedicated AV pool; proj/outproj blocks are interleaved into the
    attention stream as PE filler so exp (ACT) lag never idles the PE
"""

import sys

if "/opt/trn_rl_repo" not in sys.path:
    sys.path.insert(0, "/opt/trn_rl_repo")

from collections import deque

import numpy as np
import ml_dtypes

import concourse.bass as bass
import concourse.mybir as mybir
import concourse.tile as tile
from concourse import bacc
from concourse.bass_utils import run_bass_kernel_spmd

FP = mybir.dt.float32
BF = mybir.dt.bfloat16
NPBF = ml_dtypes.bfloat16

B, S, D, H, HD = 2, 2048, 1024, 16, 64
NCORES = 8
GH = 4  # heads per core
GW = GH * HD  # 256 qkv columns / wo rows per core
ST = 512  # s-tile
NST = S // ST
KC = 128  # k-chunk
DCH = D // 128  # 8 contraction chunks
NEG = -1.0e5
NWARM = 20  # PE warmup matmuls during initial DMA


def _emit(nc, tc, xT, wq, wk, wv, wo, cosd, sind, rotm, trim, iden, out, dbg=None):
    Exp = mybir.ActivationFunctionType.Exp
    PS = bass.MemorySpace.PSUM
    with (
        tc.tile_pool(name="const", bufs=1) as cpool,
        tc.tile_pool(name="wts", bufs=1) as wpool,
        tc.tile_pool(name="qkv", bufs=1) as qpool,
        tc.tile_pool(name="xin", bufs=4) as xpool,
        tc.tile_pool(name="wrk", bufs=3) as wrk,
        tc.tile_pool(name="psS", bufs=3, space=PS) as psS,
        tc.tile_pool(name="psA", bufs=2, space=PS) as psA,
    ):
        # ---------------- DMA: weights/x on sync+gpsimd, consts on scalar --
        xTr = xT.ap().rearrange("(c p) s -> p c s", p=128)

        def load_xt(st, q0, q1):
            t = xpool.tile([128, DCH, ST], BF, tag="xt")
            hl = DCH // 2
            ssl = slice(st * ST, (st + 1) * ST)
            q0.dma_start(t[:, 0:hl], xTr[:, 0:hl, ssl])
            q1.dma_start(t[:, hl:DCH], xTr[:, hl:DCH, ssl])
            return t

        # startup-critical tensors split across sync+gpsimd; consts on scalar
        def load_w(dram, nm):
            t = wpool.tile([128, DCH, GW], BF, name=nm, tag=nm)
            nc.sync.dma_start(t[:, 0 : DCH // 2], dram.ap()[:, 0 : DCH // 2])
            nc.gpsimd.dma_start(t[:, DCH // 2 : DCH], dram.ap()[:, DCH // 2 : DCH])
            return t

        wq_sb = load_w(wq, "wq_sb")
        xts = [None] * NST
        xts[0] = load_xt(0, nc.sync, nc.gpsimd)
        wk_sb = load_w(wk, "wk_sb")
        wv_sb = load_w(wv, "wv_sb")
        rot_sb = cpool.tile([128, 128], BF)
        nc.scalar.dma_start(rot_sb[:], rotm.ap())
        cos_sb = cpool.tile([128, S], BF)
        nc.scalar.dma_start(cos_sb[:], cosd.ap())
        sin_sb = cpool.tile([128, S], BF)
        nc.scalar.dma_start(sin_sb[:], sind.ap())
        tri_sb = cpool.tile([128, 128], BF)
        nc.scalar.dma_start(tri_sb[:], trim.ap())
        id_sb = cpool.tile([128, 128], BF)
        nc.scalar.dma_start(id_sb[:], iden.ap())
        xts[1] = load_xt(1, nc.sync, nc.gpsimd)
        wo_sb = wpool.tile([128, 2, D], BF)
        nc.scalar.dma_start(wo_sb[:], wo.ap())
        xts[2] = load_xt(2, nc.sync, nc.gpsimd)
        xts[3] = load_xt(3, nc.sync, nc.gpsimd)

        # ---------------- persistent activations ----------------
        QTt = qpool.tile([128, 2, S], BF)  # roped Q^T (ch = head pair)
        KTt = qpool.tile([128, 2, S], BF)
        Vt = qpool.tile([128, S // KC, GH * KC], BF)  # per head: 64 V | 64 ones
        attT = qpool.tile([128, 2, S], BF)

        warm = cpool.tile([128, ST], BF)
        nc.vector.memset(warm[:], 0.0)
        nc.vector.memset(Vt[:], 1.0)  # ones cols stay 1.0; V halves overwritten

        # ---------------- PE warmup (bridges initial DMA wait) -----------
        def wu(n):
            for _ in range(n):
                wps = psS.tile([128, 2, ST], FP, tag="pair", name="wps")
                nc.tensor.matmul(
                    wps[:, 0, :], warm[:, 0:128], warm[:], start=True, stop=True
                )

        wu(NWARM)

        # ---------------- emitters ----------------
        def proj_qk_ch(wsb, xt, ps, ch):
            for dc in range(DCH):
                nc.tensor.matmul(
                    ps[:, ch, :],
                    wsb[:, dc, ch * 128 : (ch + 1) * 128],
                    xt[:, dc, :],
                    start=(dc == 0),
                    stop=(dc == DCH - 1),
                )

        def rope_raw(ps):
            raw = wrk.tile([128, 2, ST], BF, tag="raw")
            nc.vector.tensor_copy(raw[:], ps[:])
            return raw

        def rope_fin(ps, raw, st, dst):
            ssl = slice(st * ST, (st + 1) * ST)
            t1 = wrk.tile([128, 2, ST], BF, tag="t1")
            for ch in range(2):
                nc.vector.tensor_mul(t1[:, ch], ps[:, ch], cos_sb[:, ssl])
            rps = psS.tile([128, 2, ST], FP, tag="pair")
            for ch in range(2):
                nc.tensor.matmul(rps[:, ch, :], rot_sb[:], raw[:, ch, :], start=True, stop=True)
            t2 = wrk.tile([128, 2, ST], BF, tag="t2")
            for ch in range(2):
                nc.vector.tensor_mul(t2[:, ch], rps[:, ch], sin_sb[:, ssl])
            nc.vector.tensor_add(dst[:, :, ssl], t1[:], t2[:])

        def vproj2(xt, st, tbpair):
            ps = psS.tile([128, 2, ST], FP, tag="pair")
            for i in range(2):
                tb = 2 * tbpair + i
                for dc in range(DCH):
                    nc.tensor.matmul(
                        ps[:, i, 0:GW],
                        xt[:, dc, tb * 128 : (tb + 1) * 128],
                        wv_sb[:, dc, :],
                        start=(dc == 0),
                        stop=(dc == DCH - 1),
                    )
                kc = st * 4 + tb
                src = ps[:, i, 0:GW].rearrange("p (h c) -> p h c", c=HD)
                # ones-first head blocks: cols 0:64 stay 1.0 (denominator rows
                # land on partition base 0 for reciprocal_approx_fast)
                dst = Vt[:, kc].rearrange("p (h c) -> p h c", c=KC)[:, :, HD:KC]
                nc.vector.tensor_copy(dst, src)

        def score_pair(h, qt, kc0):
            ch, r0 = h // 2, (h % 2) * HD
            qs = qt * ST
            sps = psS.tile([128, 2, ST], FP, tag="pair")
            xoffs = []
            for i, kc in enumerate((kc0, kc0 + 1)):
                ks = kc * KC
                off = ks - qs
                xoff = max(0, off)
                xoffs.append(xoff)
                nc.tensor.matmul(
                    sps[:, i, xoff:ST],
                    KTt[r0 : r0 + HD, ch, ks : ks + KC],
                    QTt[r0 : r0 + HD, ch, qs + xoff : qs + ST],
                    start=True,
                    stop=(off < 0),
                )
                if off >= 0:  # diagonal chunk: additive tri mask via PE
                    nc.tensor.matmul(
                        sps[:, i, xoff : xoff + KC],
                        id_sb[:],
                        tri_sb[:],
                        start=False,
                        stop=True,
                    )
            pt = wrk.tile([128, 2, ST], BF, tag="pt")
            s2 = sps[:].rearrange("p a b -> p (a b)")
            p2 = pt[:].rearrange("p a b -> p (a b)")
            nc.scalar.activation(
                p2[:, xoffs[0] : 2 * ST], s2[:, xoffs[0] : 2 * ST], Exp, scale=0.125
            )
            return pt, xoffs

        def av_pair(aps, h, kc0, pt, xoffs, nkc):
            for i, kc in enumerate((kc0, kc0 + 1)):
                xoff = xoffs[i]
                nc.tensor.matmul(
                    aps[:, xoff:ST],
                    Vt[:, kc, h * KC : (h + 1) * KC],
                    pt[:, i, xoff:ST],
                    start=(kc == 0),
                    stop=(kc == nkc - 1),
                )

        def attn_head(st, h, pop, quota):
            qt = st
            qs = qt * ST
            nkc = (qs + ST) // KC
            ch, r0 = h // 2, (h % 2) * HD
            aps = psA.tile([128, ST], FP, tag="av")
            prev = None
            cnt = 0
            for j, kc0 in enumerate(range(0, nkc, 2)):
                cur = score_pair(h, qt, kc0)
                if prev is not None:
                    av_pair(aps, h, prev[0], prev[1], prev[2], nkc)
                prev = (kc0, cur[0], cur[1])
                if j % 2 == 1 and cnt < quota:
                    pop()
                    cnt += 1
            av_pair(aps, h, prev[0], prev[1], prev[2], nkc)
            # ones-first Vt blocks: denominator = aps rows 0:63 (base 0),
            # attention values = rows 64:127
            rec = wrk.tile([64, ST], FP, tag="rec")
            nc.vector.reciprocal_approx_fast(rec[:], aps[0:64, :])
            nc.vector.tensor_mul(
                attT[r0 : r0 + HD, ch, qs : qs + ST], aps[64:128, :], rec[:]
            )
            while cnt < quota:  # leftover filler lands after the finish chain
                pop()
                cnt += 1

        def outproj_qb(qb, tail=False):
            ops = psS.tile([128, 2, ST], FP, tag="pair")
            for half in range(2):
                for ch2 in range(2):
                    nc.tensor.matmul(
                        ops[:, half, :],
                        attT[:, ch2, qb * KC : (qb + 1) * KC],
                        wo_sb[:, ch2, half * ST : (half + 1) * ST],
                        start=(ch2 == 0),
                        stop=(ch2 == 1),
                    )
            ob = wrk.tile([128, 2, ST], BF, tag="ob")
            if tail and qb % 2:  # tail: split evac across ACT+DVE
                nc.scalar.copy(ob[:], ops[:])
            else:
                nc.vector.tensor_copy(ob[:], ops[:])
            q = nc.gpsimd if qb % 2 else nc.sync
            q.dma_start(
                out[qb * KC : (qb + 1) * KC, :], ob[:].rearrange("p a b -> p (a b)")
            )

        def proj_blocks(st):
            xt = xts[st]
            ps_q, ps_k = [None], [None]
            raw_q, raw_k = [None], [None]

            def qk_first(ps_box, wsb):
                def f():
                    ps_box[0] = psS.tile([128, 2, ST], FP, tag="pair", name="ps_qk")
                    proj_qk_ch(wsb, xt, ps_box[0], 0)

                return f

            def qk_second(ps_box, wsb, raw_box):
                def f():
                    proj_qk_ch(wsb, xt, ps_box[0], 1)
                    raw_box[0] = rope_raw(ps_box[0])  # ACT copy issues early

                return f

            # raw copy (ACT) is emitted with the ch1 matmuls; the rot matmul
            # block comes two blocks later so the PE never waits on ACT.
            return [
                qk_first(ps_q, wq_sb),
                qk_second(ps_q, wq_sb, raw_q),
                qk_first(ps_k, wk_sb),
                lambda: rope_fin(ps_q[0], raw_q[0], st, QTt),
                qk_second(ps_k, wk_sb, raw_k),
                lambda: vproj2(xt, st, 0),
                lambda: rope_fin(ps_k[0], raw_k[0], st, KTt),
                lambda: vproj2(xt, st, 1),
            ]

        # ---------------- schedule ----------------
        # prologue: P(0) straight, then A(st) with interleaved fillers:
        #   A(0): P(1);  A(1): P(2)+O(0);  A(2): P(3)+O(1);  A(3): O(2)
        # epilogue: O(3)
        # P(0) with warmup matmuls masking the wk/wv DMA arrival
        pb0 = proj_blocks(0)
        pb0[0]()  # Q ch0
        pb0[1]()  # Q ch1 + raw_q
        wu(4)
        pb0[2]()  # K ch0
        pb0[3]()  # rope_fin q
        wu(4)
        for blk in pb0[4:]:
            blk()

        for st in range(NST):
            fillers = deque()
            pblk = proj_blocks(st + 1) if st + 1 < NST else []
            oblk = (
                [lambda qb=qb: outproj_qb(qb) for qb in range((st - 1) * 4, st * 4)]
                if st >= 1
                else []
            )
            # interleave: two proj blocks, then one outproj block
            pi = oi = 0
            while pi < len(pblk) or oi < len(oblk):
                for _ in range(2):
                    if pi < len(pblk):
                        fillers.append(pblk[pi])
                        pi += 1
                if oi < len(oblk):
                    fillers.append(oblk[oi])
                    oi += 1

            def pop():
                if fillers:
                    fillers.popleft()()

            # even per-head filler quotas: FIFO-on-demand starves the last
            # heads of each s-tile (h3 got zero filler and ran exp-paced)
            nf = len(fillers)
            for h in range(GH):
                q = (nf * (h + 1)) // GH - (nf * h) // GH
                attn_head(st, h, pop, q)
            while fillers:
                fillers.popleft()()

        for qb in range((NST - 1) * 4, NST * 4):
            outproj_qb(qb, tail=True)

        if dbg is not None:
            nc.sync.dma_start(dbg["QTt"].ap(), QTt[:].rearrange("p a b -> p (a b)"))
            nc.sync.dma_start(dbg["KTt"].ap(), KTt[:].rearrange("p a b -> p (a b)"))
            nc.sync.dma_start(dbg["Vt"].ap(), Vt[:].rearrange("p a b -> p (a b)"))
            nc.sync.dma_start(dbg["attT"].ap(), attT[:].rearrange("p a b -> p (a b)"))


_prog = None


def _build(with_dbg=False):
    global _prog
    if _prog is not None and not with_dbg:
        return _prog
    nc = bacc.Bacc("TRN2", target_bir_lowering=False, debug=False)
    xT = nc.declare_dram_parameter("xT", [D, S], BF, isOutput=False)
    wq = nc.declare_dram_parameter("wq", [128, DCH, GW], BF, isOutput=False)
    wk = nc.declare_dram_parameter("wk", [128, DCH, GW], BF, isOutput=False)
    wv = nc.declare_dram_parameter("wv", [128, DCH, GW], BF, isOutput=False)
    wo = nc.declare_dram_parameter("wo", [128, 2, D], BF, isOutput=False)
    cosd = nc.declare_dram_parameter("cosd", [128, S], BF, isOutput=False)
    sind = nc.declare_dram_parameter("sind", [128, S], BF, isOutput=False)
    rotm = nc.declare_dram_parameter("rotm", [128, 128], BF, isOutput=False)
    trim = nc.declare_dram_parameter("trim", [128, 128], BF, isOutput=False)
    iden = nc.declare_dram_parameter("iden", [128, 128], BF, isOutput=False)
    out = nc.declare_dram_parameter("out", [S, D], BF, isOutput=True)
    dbg = None
    if with_dbg:
        dbg = {
            "QTt": nc.declare_dram_parameter("dbg_QTt", [128, 2 * S], BF, isOutput=True),
            "KTt": nc.declare_dram_parameter("dbg_KTt", [128, 2 * S], BF, isOutput=True),
            "Vt": nc.declare_dram_parameter("dbg_Vt", [128, (S // KC) * GH * KC], BF, isOutput=True),
            "attT": nc.declare_dram_parameter("dbg_attT", [128, 2 * S], BF, isOutput=True),
        }
    with tile.TileContext(nc) as tc:
        _emit(nc, tc, xT, wq, wk, wv, wo, cosd, sind, rotm, trim, iden, out, dbg)
    nc.compile()
    if not with_dbg:
        _prog = nc
    return nc


def _tables():
    inv = 1.0 / (10000.0 ** (np.arange(0, HD, 2)[: HD // 2].astype(np.float32) / HD))
    ang = np.outer(np.arange(S, dtype=np.float32), inv).astype(np.float32)  # [S, 32]
    cos64 = np.repeat(np.cos(ang).T, 2, axis=0).astype(np.float32)  # [64, S]
    sin64 = np.repeat(np.sin(ang).T, 2, axis=0).astype(np.float32)
    cos128 = np.tile(cos64, (2, 1))
    sin128 = np.tile(sin64, (2, 1))
    rotm = np.zeros((128, 128), np.float32)
    for f in range(64):
        rotm[2 * f + 1, 2 * f] = -1.0  # out[2f]   = -x[2f+1]
        rotm[2 * f, 2 * f + 1] = 1.0  # out[2f+1] = +x[2f]
    kk, qq = np.meshgrid(np.arange(128), np.arange(128), indexing="ij")
    tri = np.where(kk <= qq, 0.0, NEG).astype(np.float32)
    iden = np.eye(128, dtype=np.float32)
    return (
        cos128.astype(NPBF),
        sin128.astype(NPBF),
        rotm.astype(NPBF),
        tri.astype(NPBF),
        iden.astype(NPBF),
    )


def _pack_w(w):  # [D, GW] -> [128, DCH, GW] bf16
    return np.ascontiguousarray(
        np.asarray(w, np.float32).reshape(DCH, 128, GW).transpose(1, 0, 2)
    ).astype(NPBF)


def _pack_wo(w):  # [GW, D] -> [128, 2, D] bf16
    return np.ascontiguousarray(
        np.asarray(w, np.float32).reshape(2, 128, D).transpose(1, 0, 2)
    ).astype(NPBF)


def make_in_maps(x, wq, wk, wv, wo_w):
    cos128, sin128, rotm, tri, iden = _tables()
    in_maps = []
    for c in range(NCORES):
        b, g = divmod(c, GH)
        cs = slice(g * GW, (g + 1) * GW)
        in_maps.append(
            {
                "xT": np.ascontiguousarray(np.asarray(x[b], np.float32).T).astype(NPBF),
                "wq": _pack_w(wq[:, cs]),
                "wk": _pack_w(wk[:, cs]),
                "wv": _pack_w(wv[:, cs]),
                "wo": _pack_wo(wo_w[cs, :]),
                "cosd": cos128,
                "sind": sin128,
                "rotm": rotm,
                "trim": tri,
                "iden": iden,
            }
        )
    return in_maps


def kernel(x, wq, wk, wv, wo_w, wo_b):
    nc = _build()
    in_maps = make_in_maps(x, wq, wk, wv, wo_w)
    res = run_bass_kernel_spmd(nc, in_maps, list(range(NCORES))).results
    out = np.zeros((B, S, D), np.float32)
    for c in range(NCORES):
        out[c // GH] += np.asarray(res[c]["out"]).astype(np.float32)
    out += np.asarray(wo_b, np.float32)[None, None, :]
    return out
